# revision 1
# baseline (speedup 1.0000x reference)
"""ResNet BasicBlock (conv3x3-BN-ReLU-conv3x3-BN-add-ReLU) on 8 Trainium2 cores.

Data-parallel over batch: 32 samples -> 4 per core. Per core, each conv is
computed as 18 shifted f32r matmuls (9 taps x 2 input-channel chunks of 128)
accumulating into PSUM; BN scale is folded into the conv weights on host,
BN bias + ReLU applied by the Scalar engine on the PSUM->SBUF pass. The
residual add runs on the Vector engine (PSUM + x -> SBUF). Images are
zero-padded to 58x58 on the host so SBUF loads are contiguous; matmul
moving operands are strided [8 rows x 56 cols] views so only real output
pixels are streamed.
"""
import os
import sys

for _p in ("/opt/trn_rl_repo", "/root/.axon_site/_ro/trn_rl_repo"):
    if os.path.isdir(_p) and _p not in sys.path:
        sys.path.append(_p)

import numpy as np

EPS = 1e-5

S = 4            # samples per core
C = 256
H = W = 56
PH = 58          # padded rows
PW = 60          # row stride: image cols at 2..57, ring zeros at cols 1,58
FLAT = PH * PW   # 3480
NCHUNK = 7       # chunks of 8 output rows
NROW = 448       # 8*56 output positions per matmul
N_CORES = 8

_CACHE = {}
LAST_RESULT = None

RESIDUAL = os.environ.get("BASS_KERNEL_RESIDUAL", "dve")  # dve | idmm
MM_DTYPE = os.environ.get("BASS_KERNEL_DTYPE", "f16")     # f16 | f32r


def _build():
    from concourse import bacc
    import concourse.mybir as mybir
    import concourse.tile as tile

    F32 = mybir.dt.float32
    F32R = mybir.dt.float32r
    F16 = mybir.dt.float16
    Relu = mybir.ActivationFunctionType.Relu

    # MMD: matmul operand dtype as stored in DRAM/SBUF. For f32r the DRAM
    # side is f32 and APs are bitcast; for f16 everything is natural fp16.
    f16 = MM_DTYPE == "f16"
    MMD = F16 if f16 else F32R
    XD = F16 if f16 else F32

    def cast(ap):
        return ap if f16 else ap.bitcast(F32R)

    nc = bacc.Bacc(None, target_bir_lowering=False)

    x_d = nc.dram_tensor("x", [S, C, PH, PW], XD, kind="ExternalInput")
    w1_d = nc.dram_tensor("w1t", [2, 128, 9, 256], XD, kind="ExternalInput")
    w2_d = nc.dram_tensor("w2t", [2, 128, 9, 256], XD, kind="ExternalInput")
    b1_d = nc.dram_tensor("b1t", [128, 2], F32, kind="ExternalInput")
    b2_d = nc.dram_tensor("b2t", [128, 2], F32, kind="ExternalInput")
    id_d = nc.dram_tensor("ident", [128, 128], XD, kind="ExternalInput")
    z_d = nc.dram_tensor("zeros", [128, 64], XD, kind="ExternalInput")
    y_d = nc.dram_tensor("y", [S, C, H, W], F32, kind="ExternalOutput")

    with tile.TileContext(nc) as tc:
        with (
            tc.tile_pool(name="wpool", bufs=1) as wpool,
            tc.tile_pool(name="img", bufs=1) as img,
            tc.tile_pool(name="outp", bufs=4) as outp,
            tc.tile_pool(name="resp", bufs=4) as resp,
            tc.tile_pool(name="ps", bufs=8, space="PSUM") as ps,
        ):
            w_sb = {}
            for conv, wd in ((1, w1_d), (2, w2_d)):
                for ci in range(2):
                    w_sb[(conv, ci)] = wpool.tile(
                        [128, 9 * 256], MMD, name=f"w{conv}_{ci}")

            def load_weights(conv, wd, taps=(0, 9)):
                # one contiguous DMA per ci chunk (dma_start issue costs ~650ns,
                # so fewer/bigger beats finer-grained); the tap range lets the
                # startup stage w1 so the first matmuls aren't gated on all taps
                t0, t1 = taps
                for ci in range(2):
                    nc.sync.dma_start(
                        w_sb[(conv, ci)][:, t0 * 256:t1 * 256],
                        cast(wd[ci, :, t0:t1, :].rearrange("p a b -> p (a b)")))

            id_t = wpool.tile([128, 128], MMD, name="id_t")
            b1_t = wpool.tile([128, 2], F32, name="b1_t")
            b2_t = wpool.tile([128, 2], F32, name="b2_t")

            xpad = {}
            o1pad = {}
            for b in range(2):
                for ci in range(2):
                    xpad[(b, ci)] = img.tile([128, FLAT], MMD, name=f"xpad{b}_{ci}")
                    o1pad[(b, ci)] = img.tile([128, FLAT], MMD, name=f"o1pad{b}_{ci}")

            def view(t):
                return t.rearrange("p (h w) -> p h w", h=PH)

            def zero_ring(t):
                v = view(t)
                zr = cast(z_d[:, 0:PW])
                zc = cast(z_d[:, 0:56])
                nc.sync.dma_start(v[:, 0:1, :], zr)
                nc.sync.dma_start(v[:, 57:58, :], zr)
                nc.sync.dma_start(v[:, 1:57, 1:2], zc)
                nc.sync.dma_start(v[:, 1:57, 58:59], zc)

            def load_sample(s, bands=((0, PH),)):
                b = s % 2
                for r0, r1 in bands:
                    for ci in range(2):
                        nc.sync.dma_start(
                            view(xpad[(b, ci)])[:, r0:r1, :],
                            cast(x_d[s, ci * 128:(ci + 1) * 128, r0:r1, :]))

            # startup: stage w1/x0 so the first chunk's matmuls gate on only
            # ~0.7MB (taps 0-2 + rows 0-9); the rest streams behind compute
            load_weights(1, w1_d, taps=(0, 3))
            load_sample(0, bands=((0, 10),))
            load_weights(1, w1_d, taps=(3, 9))
            load_sample(0, bands=((10, 29), (29, PH)))
            nc.sync.dma_start(b1_t[:, :], b1_d[:, :])
            load_weights(2, w2_d)
            if RESIDUAL == "idmm":
                nc.sync.dma_start(id_t[:, :], cast(id_d[:, :]))
            nc.sync.dma_start(b2_t[:, :], b2_d[:, :])
            for b in range(2):
                for ci in range(2):
                    zero_ring(o1pad[(b, ci)])
            load_sample(1)

            def conv_pass(conv, s, src_tiles, dst_write):
                b = s % 2
                for co in range(2):
                    for c in range(NCHUNK):
                        r0 = 1 + 8 * c
                        p = ps.tile([128, NROW], F32, name="pchunk")
                        k = 0
                        for kh in range(3):
                            for kw in range(3):
                                for ci in range(2):
                                    wofs = (kh * 3 + kw) * 256 + co * 128
                                    nc.tensor.matmul(
                                        p[:, :],
                                        w_sb[(conv, ci)][:, wofs:wofs + 128],
                                        view(src_tiles[ci])[:, r0 + kh - 1:r0 + kh + 7,
                                                            kw + 1:kw + 57],
                                        start=(k == 0),
                                        stop=(k == 17 and not
                                              (conv == 2 and RESIDUAL == "idmm")),
                                    )
                                    k += 1
                        if conv == 2 and RESIDUAL == "idmm":
                            nc.tensor.matmul(
                                p[:, :], id_t[:, :],
                                view(xpad[(b, co)])[:, r0:r0 + 8, 2:58],
                                start=False, stop=True,
                            )
                        dst_write(co, c, r0, p)

            for s in range(S):
                b = s % 2

                def write1(co, c, r0, p, b=b):
                    pout = view(o1pad[(b, co)])[:, r0:r0 + 8, 2:58]
                    nc.scalar.activation(
                        pout, p.rearrange("p (h w) -> p h w", h=8), Relu,
                        bias=b1_t[:, co:co + 1])

                conv_pass(1, s, {0: xpad[(b, 0)], 1: xpad[(b, 1)]}, write1)

                def write2(co, c, r0, p, s=s, b=b):
                    ot = outp.tile([128, NROW], F32, name="ochunk")
                    if RESIDUAL == "dve":
                        rt = resp.tile([128, NROW], F32, name="rchunk")
                        nc.vector.tensor_add(
                            rt.rearrange("p (h w) -> p h w", h=8),
                            p.rearrange("p (h w) -> p h w", h=8),
                            view(xpad[(b, co)])[:, r0:r0 + 8, 2:58]
                            if f16 else
                            view(xpad[(b, co)])[:, r0:r0 + 8, 2:58].bitcast(F32))
                        nc.scalar.activation(ot[:, :], rt[:, :], Relu,
                                             bias=b2_t[:, co:co + 1])
                    else:
                        nc.scalar.activation(ot[:, :], p[:, :], Relu,
                                             bias=b2_t[:, co:co + 1])
                    nc.sync.dma_start(
                        y_d[s, co * 128:(co + 1) * 128, r0 - 1:r0 + 7, :], ot[:, :])

                conv_pass(2, s, {0: o1pad[(b, 0)], 1: o1pad[(b, 1)]}, write2)

                if s + 2 < S:
                    load_sample(s + 2)

    nc.compile()
    return nc


def _get_nc():
    if "nc" not in _CACHE:
        _CACHE["nc"] = _build()
    return _CACHE["nc"]


def kernel(x, w1, g1, b1, m1, v1, w2, g2, b2, m2, v2):
    global LAST_RESULT
    from concourse import bass_utils

    f16 = MM_DTYPE == "f16"
    xdt = np.float16 if f16 else np.float32
    x = np.asarray(x, dtype=np.float32)
    xp = np.zeros((x.shape[0], C, PH, PW), dtype=xdt)
    xp[:, :, 1:57, 2:58] = x

    def fold(w, g, bb, m, v):
        inv = np.asarray(g, np.float64) / np.sqrt(np.asarray(v, np.float64) + EPS)
        wp = np.asarray(w, np.float64) * inv[:, None, None, None]
        bp = np.asarray(bb, np.float64) - np.asarray(m, np.float64) * inv
        wt = np.ascontiguousarray(wp.transpose(1, 2, 3, 0).reshape(2, 128, 9, 256))
        bt = np.ascontiguousarray(bp.reshape(2, 128).T)
        return wt.astype(xdt), bt.astype(np.float32)

    w1t, b1t = fold(w1, g1, b1, m1, v1)
    w2t, b2t = fold(w2, g2, b2, m2, v2)

    ident = np.eye(128, dtype=xdt)
    zeros = np.zeros((128, 64), dtype=xdt)

    nc = _get_nc()
    in_maps = []
    for c in range(N_CORES):
        in_maps.append({
            "x": np.ascontiguousarray(xp[c * S:(c + 1) * S]),
            "w1t": w1t, "w2t": w2t, "b1t": b1t, "b2t": b2t,
            "ident": ident, "zeros": zeros,
        })

    trace = bool(int(os.environ.get("BASS_KERNEL_TRACE", "0")))
    res = bass_utils.run_bass_kernel_spmd(
        nc, in_maps, core_ids=list(range(N_CORES)), trace=trace)
    LAST_RESULT = res
    out = np.concatenate([r["y"] for r in res.results], axis=0)
    return out



# revision 4
# speedup vs baseline: 1.1207x; 1.1207x over previous
"""ResNet BasicBlock (conv3x3-BN-ReLU-conv3x3-BN-add-ReLU) on 8 Trainium2 cores.

Data-parallel over batch: 32 samples -> 4 per core. Each 3x3 conv runs as a
Winograd F(2,3) transform along H (1.5x fewer PE MACs than direct conv):
rows are pre-combined on the GpSimd engine into 4 j-planes
(V0=r0-r2, V1=r1+r2, V2=r2-r1, V3=r1-r3, fp16), the PE accumulates
M_j = sum_{kw,ci} U_j[kw,ci]^T V_j(shifted kw) into 4 PSUM banks, and the
Vector engine folds the inverse transform (y_even = M0+M1+M2,
y_odd = M1-M2-M3) with one bank-spanning reduce + one scalar_tensor_tensor
per output row-pair chunk. BN scale is folded into the Winograd weights on
host; bias + ReLU run on the Scalar engine; the residual add runs on GpSimd.
Images are zero-padded to 58x58 on the host so SBUF loads are contiguous.
"""
import os
import sys

for _p in ("/opt/trn_rl_repo", "/root/.axon_site/_ro/trn_rl_repo"):
    if os.path.isdir(_p) and _p not in sys.path:
        sys.path.append(_p)

import numpy as np

EPS = 1e-5

S = 4            # samples per core
C = 256
H = W = 56
PH = 58          # padded rows (img rows -1..56)
PW = 60          # row pitch: image cols at 2..57, ring zeros at cols 1,58
FLAT = PH * PW   # 3480
T = 28           # winograd row-tiles per image (2 output rows each)
TCH = 7          # tiles per chunk
NCH = 4          # chunks (4*7 = 28 tiles)
NROW = TCH * W   # 392 moving rows per matmul
VW = 58          # V plane cols (xpad cols 1..58)
VSZ = 4 * T * VW # V plane free size per ci: j-major [4, 28, 58]
N_CORES = 8

_CACHE = {}
LAST_RESULT = None


def _build():
    from concourse import bacc
    import concourse.mybir as mybir
    import concourse.tile as tile

    F32 = mybir.dt.float32
    F16 = mybir.dt.float16
    Relu = mybir.ActivationFunctionType.Relu
    Alu = mybir.AluOpType
    AxX = mybir.AxisListType.X

    nc = bacc.Bacc(None, target_bir_lowering=False)

    x_d = nc.dram_tensor("x", [S, C, PH, PW], F16, kind="ExternalInput")
    u1_d = nc.dram_tensor("u1t", [2, 128, 24, 128], F16, kind="ExternalInput")
    u2_d = nc.dram_tensor("u2t", [2, 128, 24, 128], F16, kind="ExternalInput")
    b1_d = nc.dram_tensor("b1t", [128, 2], F32, kind="ExternalInput")
    b2_d = nc.dram_tensor("b2t", [128, 2], F32, kind="ExternalInput")
    z_d = nc.dram_tensor("zeros", [128, 64], F16, kind="ExternalInput")
    y_d = nc.dram_tensor("y", [S, C, H, W], F32, kind="ExternalOutput")

    with tile.TileContext(nc) as tc:
        with (
            tc.tile_pool(name="wpool", bufs=1) as wpool,
            tc.tile_pool(name="img", bufs=1) as img,
            tc.tile_pool(name="ep", bufs=3) as ep,
            tc.tile_pool(name="yp", bufs=3) as yp,
            tc.tile_pool(name="ps", bufs=2, space="PSUM") as ps,
        ):
            u_sb = {}
            for conv in (1, 2):
                for ci in range(2):
                    u_sb[(conv, ci)] = wpool.tile(
                        [128, 24 * 128], F16, name=f"u{conv}_{ci}")
            b1_t = wpool.tile([128, 2], F32, name="b1_t")
            b2_t = wpool.tile([128, 2], F32, name="b2_t")

            xpad = {}
            o1pad = {}
            vx = {}
            vo = {}
            for b in range(2):
                for ci in range(2):
                    xpad[(b, ci)] = img.tile([128, FLAT], F16, name=f"xpad{b}_{ci}")
                    o1pad[(b, ci)] = img.tile([128, FLAT], F16, name=f"o1pad{b}_{ci}")
                    vx[(b, ci)] = img.tile([128, VSZ], F16, name=f"vx{b}_{ci}")
            for ci in range(2):
                vo[ci] = img.tile([128, VSZ], F16, name=f"vo_{ci}")

            def view(t):
                return t.rearrange("p (h w) -> p h w", h=PH)

            def view2(t):
                # [p, 29, 2, 60]: row 2t+a at [:, t, a, :]
                return t.rearrange("p (t two w) -> p t two w", two=2, w=PW)

            def vview(t):
                return t.rearrange("p (j t w) -> p j t w", j=4, t=T)

            def load_weights(conv, ud, blks=(0, 24)):
                k0, k1 = blks
                for ci in range(2):
                    nc.sync.dma_start(
                        u_sb[(conv, ci)][:, k0 * 128:k1 * 128],
                        ud[ci, :, k0:k1, :].rearrange("p a b -> p (a b)"))

            def load_sample(s, bands=((0, PH),)):
                b = s % 2
                for r0, r1 in bands:
                    for ci in range(2):
                        nc.sync.dma_start(
                            view(xpad[(b, ci)])[:, r0:r1, :],
                            x_d[s, ci * 128:(ci + 1) * 128, r0:r1, :])

            def zero_ring(t):
                v = view(t)
                nc.sync.dma_start(v[:, 0:1, :], z_d[:, 0:PW])
                nc.sync.dma_start(v[:, 57:58, :], z_d[:, 0:PW])
                nc.sync.dma_start(v[:, 1:57, 1:2], z_d[:, 0:56])
                nc.sync.dma_start(v[:, 1:57, 58:59], z_d[:, 0:56])

            def fwd_transform(src_tiles, dst_tiles, tr=(0, T)):
                # V0 = r0-r2, V1 = r1+r2, V2 = r2-r1, V3 = r1-r3 where
                # r_a = src row 2t+a; all reads/writes packed fp16.
                t0, t1 = tr
                for ci in range(2):
                    sv = view2(src_tiles[ci])
                    r0 = sv[:, t0:t1, 0, 1:1 + VW]
                    r1 = sv[:, t0:t1, 1, 1:1 + VW]
                    r2 = sv[:, t0 + 1:t1 + 1, 0, 1:1 + VW]
                    r3 = sv[:, t0 + 1:t1 + 1, 1, 1:1 + VW]
                    dv = vview(dst_tiles[ci])
                    nc.gpsimd.tensor_sub(dv[:, 0, t0:t1, :], r0, r2)
                    nc.gpsimd.tensor_add(dv[:, 1, t0:t1, :], r1, r2)
                    nc.gpsimd.tensor_sub(dv[:, 2, t0:t1, :], r2, r1)
                    nc.gpsimd.tensor_sub(dv[:, 3, t0:t1, :], r1, r3)

            def mm_chunk(conv, v_tiles, co, c):
                # 24 matmuls accumulating M_j into PSUM banks j=0..3
                p = ps.tile([128, 4, 512], F32, name="pj")
                t0 = TCH * c
                for j in range(4):
                    for kw in range(3):
                        for ci in range(2):
                            blk = (j * 3 + kw) * 2 + co
                            nc.tensor.matmul(
                                p[:, j, 0:NROW],
                                u_sb[(conv, ci)][:, blk * 128:(blk + 1) * 128],
                                vview(v_tiles[ci])[:, j, t0:t0 + TCH, kw:kw + W],
                                start=(kw == 0 and ci == 0),
                                stop=(kw == 2 and ci == 1),
                            )
                return p

            def inverse(p):
                # y_even = M0+M1+M2 ; y_odd = M1-M2-M3 (from 4 PSUM banks)
                er = ep.tile([128, NROW], F32, name="er")
                t23 = ep.tile([128, NROW], F32, name="t23")
                orow = ep.tile([128, NROW], F32, name="orow")
                red = p[:, :, 0:NROW].rearrange("p j n -> p n j")
                nc.vector.tensor_reduce(er[:, :], red[:, :, 0:3], AxX, Alu.add)
                nc.vector.tensor_reduce(t23[:, :], red[:, :, 2:4], AxX, Alu.add)
                nc.vector.scalar_tensor_tensor(
                    orow[:, :], p[:, 1, 0:NROW], 0.0, t23[:, :],
                    op0=Alu.bypass, op1=Alu.subtract)
                return er, orow

            def rows3(t):
                return t.rearrange("p (h w) -> p h w", h=TCH)

            # ---- startup staging ----
            load_weights(1, u1_d, blks=(0, 6))
            nc.sync.dma_start(b1_t[:, :], b1_d[:, :])
            load_sample(0, bands=((0, 16),))
            load_weights(1, u1_d, blks=(6, 24))
            load_sample(0, bands=((16, 30), (30, 44), (44, PH)))
            # per-chunk transforms for sample 0 (chunk c reads rows 14c..14c+15)
            for c in range(NCH):
                fwd_transform({ci: xpad[(0, ci)] for ci in range(2)},
                              {ci: vx[(0, ci)] for ci in range(2)},
                              tr=(TCH * c, TCH * (c + 1)))
            load_weights(2, u2_d)
            nc.sync.dma_start(b2_t[:, :], b2_d[:, :])
            for b in range(2):
                for ci in range(2):
                    zero_ring(o1pad[(b, ci)])
            load_sample(1)
            fwd_transform({ci: xpad[(1, ci)] for ci in range(2)},
                          {ci: vx[(1, ci)] for ci in range(2)})

            for s in range(S):
                b = s % 2

                # conv1: x -> o1 (via vx), bias+relu on scalar into o1pad
                for c in range(NCH):
                    for co in range(2):
                        p = mm_chunk(1, {ci: vx[(b, ci)] for ci in range(2)}, co, c)
                        er, orow = inverse(p)
                        ov = view2(o1pad[(b, co)])
                        # img even rows 2t -> pad row 2t+1; odd 2t+1 -> 2t+2
                        nc.scalar.activation(
                            ov[:, 7 * c:7 * c + 7, 1, 2:58], rows3(er), Relu,
                            bias=b1_t[:, co:co + 1])
                        nc.scalar.activation(
                            ov[:, 7 * c + 1:7 * c + 8, 0, 2:58], rows3(orow), Relu,
                            bias=b1_t[:, co:co + 1])
                    if c >= 1:
                        fwd_transform({ci: o1pad[(b, ci)] for ci in range(2)},
                                      vo, tr=(TCH * (c - 1), TCH * c))
                fwd_transform({ci: o1pad[(b, ci)] for ci in range(2)},
                              vo, tr=(TCH * 3, TCH * 4))

                # conv2: o1 -> y (via vo), residual add on gpsimd, bias+relu scalar
                for c in range(NCH):
                    for co in range(2):
                        p = mm_chunk(2, vo, co, c)
                        er, orow = inverse(p)
                        xv = view2(xpad[(b, co)])
                        ea = ep.tile([128, NROW], F32, name="ea")
                        oa = ep.tile([128, NROW], F32, name="oa")
                        nc.gpsimd.tensor_add(
                            rows3(ea), rows3(er),
                            xv[:, 7 * c:7 * c + 7, 1, 2:58])
                        nc.gpsimd.tensor_add(
                            rows3(oa), rows3(orow),
                            xv[:, 7 * c + 1:7 * c + 8, 0, 2:58])
                        ys = yp.tile([128, 2 * NROW], F32, name="ys")
                        yv = ys.rearrange("p (t two w) -> p t two w", two=2, w=W)
                        nc.scalar.activation(
                            yv[:, :, 0, :], rows3(ea), Relu,
                            bias=b2_t[:, co:co + 1])
                        nc.scalar.activation(
                            yv[:, :, 1, :], rows3(oa), Relu,
                            bias=b2_t[:, co:co + 1])
                        nc.sync.dma_start(
                            y_d[s, co * 128:(co + 1) * 128, 14 * c:14 * c + 14, :],
                            ys[:, :])

                if s + 2 < S:
                    load_sample(s + 2)
                    fwd_transform({ci: xpad[(b, ci)] for ci in range(2)},
                                  {ci: vx[(b, ci)] for ci in range(2)})

    nc.compile()
    return nc


def _get_nc():
    if "nc" not in _CACHE:
        _CACHE["nc"] = _build()
    return _CACHE["nc"]


G_WINO = np.array([[1, 0, 0], [0.5, 0.5, 0.5], [0.5, -0.5, 0.5], [0, 0, 1]],
                  dtype=np.float64)


def kernel(x, w1, g1, b1, m1, v1, w2, g2, b2, m2, v2):
    global LAST_RESULT
    from concourse import bass_utils

    x = np.asarray(x, dtype=np.float32)
    xp = np.zeros((x.shape[0], C, PH, PW), dtype=np.float16)
    xp[:, :, 1:57, 2:58] = x

    def fold(w, g, bb, m, v):
        inv = np.asarray(g, np.float64) / np.sqrt(np.asarray(v, np.float64) + EPS)
        wp = np.asarray(w, np.float64) * inv[:, None, None, None]
        bp = np.asarray(bb, np.float64) - np.asarray(m, np.float64) * inv
        # U[j, kw][ic, oc] = sum_kh G[j, kh] * wp[oc, ic, kh, kw]
        U = np.einsum('jk,oikw->jwio', G_WINO, wp)   # [4, 3, I, O]
        ut = np.zeros((2, 128, 24, 128), dtype=np.float16)
        for j in range(4):
            for kw in range(3):
                for co in range(2):
                    blk = (j * 3 + kw) * 2 + co
                    for ci in range(2):
                        ut[ci, :, blk, :] = U[j, kw, ci * 128:(ci + 1) * 128,
                                              co * 128:(co + 1) * 128]
        bt = np.ascontiguousarray(bp.reshape(2, 128).T).astype(np.float32)
        return ut, bt

    u1t, b1t = fold(w1, g1, b1, m1, v1)
    u2t, b2t = fold(w2, g2, b2, m2, v2)

    zeros = np.zeros((128, 64), dtype=np.float16)

    nc = _get_nc()
    in_maps = []
    for c in range(N_CORES):
        in_maps.append({
            "x": np.ascontiguousarray(xp[c * S:(c + 1) * S]),
            "u1t": u1t, "u2t": u2t, "b1t": b1t, "b2t": b2t,
            "zeros": zeros,
        })

    trace = bool(int(os.environ.get("BASS_KERNEL_TRACE", "0")))
    res = bass_utils.run_bass_kernel_spmd(
        nc, in_maps, core_ids=list(range(N_CORES)), trace=trace)
    LAST_RESULT = res
    out = np.concatenate([r["y"] for r in res.results], axis=0)
    return out
